# revision 20
# baseline (speedup 1.0000x reference)
"""AlexNet + top-1 MoE head, data-parallel over 8 TRN2 NeuronCores.

Self-contained: takes FULL inputs, shards batch 32/core, runs one SPMD
Bass kernel on cores 0-7, gathers [256, 100] output.
"""
import os
import ml_dtypes
import numpy as np

import concourse.bass as bass
import concourse.mybir as mybir
import concourse.tile as tile
from concourse import bacc
from concourse.bass_utils import run_bass_kernel_spmd
from concourse.masks import make_identity

F32 = mybir.dt.float32
F32R = mybir.dt.float32r
BF16 = mybir.dt.bfloat16
F16 = mybir.dt.float16

B = 256
NCORE = 8
BL = B // NCORE  # 32 images per core
BN_EPS = 1e-5

# ---------------------------------------------------------------------------
# host-side parameter/layout preparation (numpy only, no arithmetic on acts)
# ---------------------------------------------------------------------------


def _np(x):
    return np.asarray(x, dtype=np.float32)


def prep_host(inputs):
    conv_ws = [_np(w) for w in inputs["conv_ws"]]
    conv_bs = [_np(b) for b in inputs["conv_bs"]]
    bn_g = [_np(v) for v in inputs["bn_g"]]
    bn_b = [_np(v) for v in inputs["bn_b"]]
    bn_m = [_np(v) for v in inputs["bn_m"]]
    bn_v = [_np(v) for v in inputs["bn_v"]]

    w_eff, b_eff = [], []
    for i in range(5):
        sc = bn_g[i] / np.sqrt(bn_v[i] + BN_EPS)
        w_eff.append(conv_ws[i] * sc[:, None, None, None])
        b_eff.append(conv_bs[i] * sc + bn_b[i] - bn_m[i] * sc)

    out = {}

    # ---- conv1: phase decomposition of the stride-4 11x11 conv ----
    x = _np(inputs["x"])  # [B, 3, 227, 227]
    xp = np.zeros((B, 3, 228, 228), np.float32)
    xp[:, :, :227, :227] = x
    # phase planes: [B, ci, q, r, 57, 57]; partition index ciqr = ci*16+q*4+r
    ph = xp.reshape(B, 3, 57, 4, 57, 4).transpose(0, 1, 3, 5, 2, 4)
    php = np.zeros((B, 48, 3252), np.float32)
    php[:, :, :3249] = ph.reshape(B, 48, 3249)
    # replicated K-chunk layout: rows 0..127 = chunk0 (b0 all, b1 all, b2
    # ciqr 0..31); rows 128..175 = a-shifted b2 remainder (a*16 + (ciqr-32))
    xr = np.empty((B, 176, 3135), np.float32)
    xr[:, 0:48] = php[:, :, 0:3135]
    xr[:, 48:96] = php[:, :, 57:3192]
    xr[:, 96:128] = php[:, 0:32, 114:3249]
    for a in range(3):
        xr[:, 128 + 16 * a : 144 + 16 * a] = php[:, 32:48, 114 + a : 3249 + a]
    out["xp"] = xr.astype(np.float16)

    # w1 phase lhsT: [144 (b*48 + ciqr), 3 (a), 96 (co)]
    w1p = np.zeros((3, 48, 3, 96), np.float32)  # [b, ciqr, a, co]
    for b in range(3):
        for ci in range(3):
            for q in range(4):
                for r in range(4):
                    ky = 4 * b + q
                    if ky > 10:
                        continue
                    for a in range(3):
                        kx = 4 * a + r
                        if kx > 10:
                            continue
                        w1p[b, ci * 16 + q * 4 + r, a, :] = w_eff[0][:, ci, ky, kx]
    w1p = w1p.reshape(144, 3, 96)
    out["w1a"] = np.ascontiguousarray(w1p[0:128]).astype(np.float16)
    w1c = np.zeros((48, 96), np.float32)
    for a in range(3):
        w1c[a * 16 : (a + 1) * 16] = w1p[128:144, a, :]
    out["w1c"] = w1c.astype(np.float16)
    out["b1"] = b_eff[0].reshape(96, 1)

    # ---- conv2 lhsT: [480 (ky*96+ci), 5 (kx), 256 (co)] ----
    # w_eff[1]: [256, 96, 5, 5] -> [ky, ci, kx, co]
    out["w2p"] = np.ascontiguousarray(
        w_eff[1].transpose(2, 1, 3, 0).reshape(480, 5, 256)
    ).astype(np.float16)
    out["b2"] = b_eff[1].reshape(256, 1)

    # ---- conv3-5 lhsT: [ci, 9 (ky*3+kx), co] ----
    for i, nm in ((2, "w3p"), (3, "w4p"), (4, "w5p")):
        w = w_eff[i]  # [co, ci, 3, 3]
        out[nm] = np.ascontiguousarray(
            w.transpose(1, 2, 3, 0).reshape(w.shape[1], 9, w.shape[0])
        ).astype(np.float16)
    out["b3"] = b_eff[2].reshape(384, 1)
    out["b4"] = b_eff[3].reshape(384, 1)
    out["b5"] = b_eff[4].reshape(256, 1)

    # ---- fc1: K-order (cc, s, p): chunk k = cc*36+s, partition p = ch%128 ----
    fc1_w = _np(inputs["fc1_w"])  # [4096, 9216]
    w1s = fc1_w.T.reshape(2, 128, 36, 4096).transpose(0, 2, 1, 3)
    out["w1s"] = np.ascontiguousarray(w1s.reshape(72, 128, 4096)).astype(
        np.float16)
    out["fb1"] = np.broadcast_to(_np(inputs["fc1_b"]), (BL, 4096)).copy()

    fc2_w = _np(inputs["fc2_w"])  # [4096, 4096]
    out["w2s"] = np.ascontiguousarray(fc2_w.T.reshape(32, 128, 4096)).astype(
        np.float16)
    out["fb2"] = np.broadcast_to(_np(inputs["fc2_b"]), (BL, 4096)).copy()

    # ---- gate (padded to 4 experts) ----
    gate_w = _np(inputs["gate_w"])  # [3, 4096]
    gw4 = np.zeros((4, 4096), np.float32)
    gw4[:3] = gate_w
    out["gw"] = np.ascontiguousarray(gw4.T.reshape(32, 128, 4)).astype(np.float16)
    gb4 = np.full((4,), -1e30, np.float32)
    gb4[:3] = _np(inputs["gate_b"]) + _np(inputs["expert_bias"])
    out["gb"] = np.broadcast_to(gb4, (BL, 4)).copy()

    # ---- experts ----
    exp_w = _np(inputs["exp_w"])  # [3, 4096, 100]
    out["ew"] = np.ascontiguousarray(
        exp_w.transpose(1, 0, 2).reshape(32, 128, 300)
    ).astype(np.float16)
    out["eb"] = np.broadcast_to(
        _np(inputs["exp_b"]).reshape(300), (BL, 300)
    ).copy()
    return out


# ---------------------------------------------------------------------------
# device kernel
# ---------------------------------------------------------------------------


def memz(nc, ap):
    if mybir.dt.size(ap.dtype) == 2:
        nc.vector.memset(ap.bitcast(mybir.dt.uint16), 0)
    else:
        nc.vector.memset(ap.bitcast(mybir.dt.uint32), 0)


def rview(t, off, dims):
    """Strided free-dim view of an SBUF tile AP (keeps its partition dim)."""
    base = t if isinstance(t, bass.AP) else t[:]
    part = list(base.ap[0])
    return bass.AP(base.tensor, base.offset + off, [part] + [list(d) for d in dims])


def build_nc(n_img=BL):
    nc = bacc.Bacc("TRN2", target_bir_lowering=False, debug=False)
    AF = mybir.ActivationFunctionType

    xp_d = nc.dram_tensor("xp", [n_img, 176, 3135], F16, kind="ExternalInput")
    w1a_d = nc.dram_tensor("w1a", [128, 3, 96], F16, kind="ExternalInput")
    w1c_d = nc.dram_tensor("w1c", [48, 96], F16, kind="ExternalInput")
    b1_d = nc.dram_tensor("b1", [96, 1], F32, kind="ExternalInput")
    w2p_d = nc.dram_tensor("w2p", [480, 5, 256], F16, kind="ExternalInput")
    b2_d = nc.dram_tensor("b2", [256, 1], F32, kind="ExternalInput")
    w3p_d = nc.dram_tensor("w3p", [256, 9, 384], F16, kind="ExternalInput")
    b3_d = nc.dram_tensor("b3", [384, 1], F32, kind="ExternalInput")
    w4p_d = nc.dram_tensor("w4p", [384, 9, 384], F16, kind="ExternalInput")
    b4_d = nc.dram_tensor("b4", [384, 1], F32, kind="ExternalInput")
    w5p_d = nc.dram_tensor("w5p", [384, 9, 256], F16, kind="ExternalInput")
    b5_d = nc.dram_tensor("b5", [256, 1], F32, kind="ExternalInput")
    w1s_d = nc.dram_tensor("w1s", [72, 128, 4096], F16, kind="ExternalInput")
    fb1_d = nc.dram_tensor("fb1", [n_img, 4096], F32, kind="ExternalInput")
    w2s_d = nc.dram_tensor("w2s", [32, 128, 4096], F16, kind="ExternalInput")
    fb2_d = nc.dram_tensor("fb2", [n_img, 4096], F32, kind="ExternalInput")
    gw_d = nc.dram_tensor("gw", [32, 128, 4], F16, kind="ExternalInput")
    gb_d = nc.dram_tensor("gb", [n_img, 4], F32, kind="ExternalInput")
    ew_d = nc.dram_tensor("ew", [32, 128, 300], F16, kind="ExternalInput")
    eb_d = nc.dram_tensor("eb", [n_img, 300], F32, kind="ExternalInput")
    out_d = nc.dram_tensor("out", [n_img, 100], F32, kind="ExternalOutput")

    act1_d = nc.dram_tensor("act1", [n_img, 96, 961], F16)
    act2_d = nc.dram_tensor("act2", [n_img, 256, 225], F16)
    act3_d = nc.dram_tensor("act3", [n_img, 384, 225], F16)
    act4_d = nc.dram_tensor("act4", [n_img, 384, 225], F16)

    assert n_img % 2 == 0
    npair = n_img // 2

    with tile.TileContext(nc) as tc:
        with tc.tile_pool(name="persist", bufs=1) as pers:
            # features [128, cc, s, img] gathered across the conv phase
            feat = pers.tile([128, 2, 36, n_img], F16)

            # all conv weights+biases preloaded on the gpsimd DMA queue so
            # they never stall the sync queue at phase boundaries
            cw = pers
            w1a = cw.tile([128, 3, 96], F16)
            nc.gpsimd.dma_start(w1a[:], w1a_d[:])
            w1c = cw.tile([48, 96], F16)
            nc.gpsimd.dma_start(w1c[:], w1c_d[:])
            b1 = cw.tile([96, 1], F32)
            nc.gpsimd.dma_start(b1[:], b1_d[:])
            w2c = []
            for t in range(4):
                kk = 128 if t < 3 else 96
                w2t = cw.tile([kk, 5, 256], F16, tag=f"w2_{t}", name=f"w2_{t}")
                nc.gpsimd.dma_start(w2t[:], w2p_d[t * 128 : t * 128 + kk])
                w2c.append(w2t)
            b2a = cw.tile([128, 1], F32)
            nc.gpsimd.dma_start(b2a[:], b2_d[0:128])
            b2b = cw.tile([128, 1], F32)
            nc.gpsimd.dma_start(b2b[:], b2_d[128:256])

            def load_w33(w_d, b_d, nci, nco, pfx):
                wts, bs = [], []
                for t in range(nci // 128):
                    wt = cw.tile([128, 9, nco], F16, tag=f"{pfx}w_{t}",
                                 name=f"{pfx}w_{t}")
                    nc.gpsimd.dma_start(wt[:], w_d[t * 128 : (t + 1) * 128])
                    wts.append(wt)
                for c in range(nco // 128):
                    bt = cw.tile([128, 1], F32, tag=f"{pfx}b_{c}", name=f"{pfx}b_{c}")
                    nc.gpsimd.dma_start(bt[:], b_d[c * 128 : (c + 1) * 128])
                    bs.append(bt)
                return wts, bs

            w3t, b3t = load_w33(w3p_d, b3_d, 256, 384, "c3")
            w4t, b4t = load_w33(w4p_d, b4_d, 384, 384, "c4")
            w5t, b5t = load_w33(w5p_d, b5_d, 384, 256, "c5")

            gw = cw.tile([128, 32, 4], F16)
            nc.gpsimd.dma_start(gw[:], gw_d[:].rearrange("k p e -> p k e"))
            ew = cw.tile([128, 32, 300], F16)
            nc.gpsimd.dma_start(ew[:], ew_d[:].rearrange("k p e -> p k e"))
            gb = cw.tile([n_img, 4], F32)
            nc.gpsimd.dma_start(gb[:], gb_d[:])
            eb = cw.tile([n_img, 300], F32)
            nc.gpsimd.dma_start(eb[:], eb_d[:])

            # ============ conv1 ============
            with tc.tile_pool(name="c1x", bufs=3) as xpool, \
                 tc.tile_pool(name="c1o", bufs=2) as opool, \
                 tc.tile_pool(name="c1ps", bufs=4, space="PSUM") as pp:
                for im in range(n_img):
                    x1a = xpool.tile([128, 3152], F16, tag="x1a")
                    x1c = xpool.tile([48, 3152], F16, tag="x1c")
                    nc.sync.dma_start(x1a[:, 0:3135], xp_d[im, 0:128])
                    nc.sync.dma_start(x1c[:, 0:3135], xp_d[im, 128:176])

                    c1o = opool.tile([96, 55, 56], F16, tag="c1o")
                    y0 = 0
                    while y0 < 55:
                        ny = min(9, 55 - y0)
                        ps = pp.tile([96, 512], F32, tag="ps", name="ps")[:, : ny * 56]
                        for a in range(3):
                            nc.tensor.matmul(
                                ps,
                                w1a[:, a, :],
                                rview(x1a, y0 * 57 + a, [[57, ny], [1, 56]]),
                                start=(a == 0), stop=False,
                            )
                        nc.tensor.matmul(
                            ps,
                            w1c[:],
                            rview(x1c, y0 * 57, [[57, ny], [1, 56]]),
                            start=False, stop=True,
                        )
                        nc.scalar.activation(
                            c1o[:, y0 : y0 + ny, :],
                            ps.rearrange("p (y x) -> p y x", y=ny),
                            AF.Relu, bias=b1[:], scale=1.0,
                        )
                        y0 += ny

                    # maxpool 3x3 s2: 55x55 -> 27x27 (col 55 of c1o is garbage)
                    t1 = opool.tile([96, 55, 27], F16, tag="t1")
                    v = c1o[:]
                    nc.vector.tensor_max(t1[:], v[:, :, 0:54:2], v[:, :, 1:55:2])
                    nc.vector.tensor_max(t1[:], t1[:], v[:, :, 2:56:2])
                    tmp2 = opool.tile([96, 27, 27], F16, tag="tmp2")
                    nc.vector.tensor_max(tmp2[:], t1[:, 0:54:2, :], t1[:, 1:55:2, :])
                    pad1 = opool.tile([96, 961], F16, tag="pad1")
                    memz(nc, pad1[:])
                    p1v = pad1[:].rearrange("p (y x) -> p y x", y=31)
                    nc.vector.tensor_max(p1v[:, 2:29, 2:29], tmp2[:], t1[:, 2:55:2, :])
                    nc.sync.dma_start(act1_d[im], pad1[:])

            # ============ conv2 ============
            # K enumeration p = ky*96 + ci over [0,480); chunks of 128.
            # sections: (chunk, dst_lo, dst_hi, ky, ci_lo, ci_hi)
            sects = [
                (0, 0, 96, 0, 0, 96), (0, 96, 128, 1, 0, 32),
                (1, 0, 64, 1, 32, 96), (1, 64, 128, 2, 0, 64),
                (2, 0, 32, 2, 64, 96), (2, 32, 128, 3, 0, 96),
                (3, 0, 96, 4, 0, 96),
            ]
            with tc.tile_pool(name="c2x", bufs=3) as xpool, \
                 tc.tile_pool(name="c2o", bufs=2) as opool, \
                 tc.tile_pool(name="c2ps", bufs=4, space="PSUM") as pp:
                for pr in range(npair):
                    im = 2 * pr
                    k2 = []
                    for t in range(4):
                        kk = 128 if t < 3 else 96
                        k2t = xpool.tile([kk, 1696], F16, tag=f"k2_{t}", name=f"k2_{t}")
                        k2.append(k2t)
                    for (t, dlo, dhi, ky, clo, chi) in sects:
                        nc.sync.dma_start(
                            rview(k2[t][dlo:dhi], 0, [[844, 2], [1, 837]]),
                            act1_d[im : im + 2, clo:chi, 31 * ky : 31 * ky + 837]
                            .rearrange("n k f -> k n f"),
                        )
                    for cc in range(2):
                        bia = b2a[:] if cc == 0 else b2b[:]
                        c2o = opool.tile([128, 2, 27, 27], F16, tag=f"c2o_{cc}",
                                         name=f"c2o_{cc}")
                        for y0 in (0, 9, 18):
                            ps = pp.tile([128, 486], F32, tag="ps", name="ps")
                            first = True
                            for t in range(4):
                                for kx in range(5):
                                    nc.tensor.matmul(
                                        ps[:],
                                        w2c[t][:, kx, cc * 128 : cc * 128 + 128],
                                        rview(k2[t], y0 * 31 + kx,
                                              [[844, 2], [31, 9], [1, 27]]),
                                        start=first,
                                        stop=(t == 3 and kx == 4),
                                    )
                                    first = False
                            nc.scalar.activation(
                                c2o[:, :, y0 : y0 + 9, :],
                                ps[:].rearrange("p (n y x) -> p n y x", n=2, y=9),
                                AF.Relu, bias=bia, scale=1.0,
                            )
                        # pool 27x27 -> 13x13 (col 27 garbage)
                        t1 = opool.tile([128, 2, 27, 13], F16, tag="p2t1")
                        v = c2o[:]
                        nc.vector.tensor_max(t1[:], v[:, :, :, 0:25:2], v[:, :, :, 1:26:2])
                        nc.vector.tensor_max(t1[:], t1[:], v[:, :, :, 2:27:2])
                        tmp2 = opool.tile([128, 2, 13, 13], F16, tag="p2tmp2")
                        nc.vector.tensor_max(tmp2[:], t1[:, :, 0:25:2, :],
                                             t1[:, :, 1:26:2, :])
                        pad2 = opool.tile([128, 2, 225], F16, tag="pad2")
                        memz(nc, pad2[:])
                        p2v = pad2[:].rearrange("p n (y x) -> p n y x", y=15)
                        nc.vector.tensor_max(p2v[:, :, 1:14, 1:14], tmp2[:],
                                             t1[:, :, 2:27:2, :])
                        nc.sync.dma_start(
                            act2_d[im : im + 2, cc * 128 : cc * 128 + 128, :]
                            .rearrange("n k f -> k n f"),
                            pad2[:],
                        )

            # ============ conv3 / conv4 / conv5 (pairs of images) ============
            def conv33(src_d, dst_d, wts, bs, nci, nco, pool5=False):
                """3x3 pad-1 conv on 15x15-padded 13x13 acts, 2 imgs/pass."""
                ncic = nci // 128
                ncoc = nco // 128
                with tc.tile_pool(name="c3x", bufs=2) as xpool, \
                     tc.tile_pool(name="c3o", bufs=2) as opool, \
                     tc.tile_pool(name="c3ps", bufs=4, space="PSUM") as pp:
                    for pr in range(npair):
                        im = 2 * pr
                        xt = []
                        for t in range(ncic):
                            x3 = xpool.tile([128, 456], F16, tag=f"x_{t}",
                                            name=f"x_{t}")
                            nc.sync.dma_start(
                                rview(x3, 0, [[226, 2], [1, 225]]),
                                src_d[im : im + 2, t * 128 : (t + 1) * 128, :]
                                .rearrange("n k f -> k n f"),
                            )
                            xt.append(x3)
                        for c in range(ncoc):
                            ps = pp.tile([128, 338], F32, tag="ps")
                            first = True
                            for t in range(ncic):
                                for ky in range(3):
                                    for kx in range(3):
                                        nc.tensor.matmul(
                                            ps[:],
                                            wts[t][:, 3 * ky + kx,
                                                   c * 128 : (c + 1) * 128],
                                            rview(xt[t], ky * 15 + kx,
                                                  [[226, 2], [15, 13], [1, 13]]),
                                            start=first,
                                            stop=(t == ncic - 1 and ky == 2
                                                  and kx == 2),
                                        )
                                        first = False
                            if not pool5:
                                pad3 = opool.tile([128, 2, 225], F16,
                                                  tag=f"pad_{c}", name=f"pad_{c}")
                                memz(nc, pad3[:])
                                p3v = pad3[:].rearrange("p n (y x) -> p n y x", y=15)
                                nc.scalar.activation(
                                    p3v[:, :, 1:14, 1:14],
                                    ps[:].rearrange("p (n y x) -> p n y x",
                                                    n=2, y=13),
                                    AF.Relu, bias=bs[c], scale=1.0,
                                )
                                nc.sync.dma_start(
                                    dst_d[im : im + 2, c * 128 : (c + 1) * 128, :]
                                    .rearrange("n k f -> k n f"),
                                    pad3[:],
                                )
                            else:
                                c5o = opool.tile([128, 2, 13, 13], F16,
                                                 tag=f"c5o_{c}", name=f"c5o_{c}")
                                nc.scalar.activation(
                                    c5o[:],
                                    ps[:].rearrange("p (n y x) -> p n y x",
                                                    n=2, y=13),
                                    AF.Relu, bias=bs[c], scale=1.0,
                                )
                                # pool 13x13 -> 6x6 into feat[:, c, :, im:im+2]
                                t1 = opool.tile([128, 2, 13, 6], F16, tag="p5t1")
                                v = c5o[:]
                                nc.vector.tensor_max(t1[:], v[:, :, :, 0:11:2],
                                                     v[:, :, :, 1:12:2])
                                nc.vector.tensor_max(t1[:], t1[:], v[:, :, :, 2:13:2])
                                tmp2 = opool.tile([128, 2, 6, 6], F16, tag="p5tmp2")
                                nc.vector.tensor_max(tmp2[:], t1[:, :, 0:11:2, :],
                                                     t1[:, :, 1:12:2, :])
                                fv = feat[:].rearrange(
                                    "p c (y x) n -> p c y x n", y=6)
                                for j in range(2):
                                    nc.vector.tensor_max(
                                        fv[:, c, :, :, im + j],
                                        tmp2[:, j], t1[:, j, 2:13:2, :]
                                    )

            conv33(act2_d, act3_d, w3t, b3t, 256, 384)
            conv33(act3_d, act4_d, w4t, b4t, 384, 384)
            conv33(act4_d, None, w5t, b5t, 384, 256, pool5=True)

            # ============ FC trunk ============
            ident = pers.tile([n_img, n_img], F32)
            make_identity(nc, ident[:])

            def fc_layer(w_stream_d, nk, lhs_get, bias_d, out_t):
                """out = relu(lhs.T @ W + b); lhs chunks [128, n_img] stationary.
                Writes transposed f32r result into out_t [128, nk_out, n_img]."""
                with tc.tile_pool(name="fcw", bufs=8) as wpool, \
                     tc.tile_pool(name="fco", bufs=1) as opool:
                    bias = opool.tile([n_img, 4096], F32)
                    nc.sync.dma_start(bias[:], bias_d[:])
                    h_sb = opool.tile([n_img, 4096], F32)
                    with tc.tile_pool(name="fcps", bufs=1, space="PSUM") as pp:
                        pss = [pp.tile([n_img, 512], F32, name=f"fps_{n}")
                               for n in range(8)]
                        for k in range(nk):
                            wt = wpool.tile([128, 4096], F16, tag="wt")
                            nc.sync.dma_start(wt[:], w_stream_d[k])
                            lhs = lhs_get(k)
                            for n in range(8):
                                nc.tensor.matmul(
                                    pss[n][:], lhs, wt[:, n * 512 : (n + 1) * 512],
                                    start=(k == 0), stop=(k == nk - 1),
                                )
                        for n in range(8):
                            sl = slice(n * 512, (n + 1) * 512)
                            nc.vector.tensor_add(h_sb[:, sl], pss[n][:], bias[:, sl])
                            nc.vector.tensor_scalar_max(h_sb[:, sl], h_sb[:, sl], 0.0)
                    with tc.tile_pool(name="fct", bufs=2, space="PSUM") as tp:
                        for c in range(32):
                            tps = tp.tile([128, n_img], F32, tag="tps")
                            nc.tensor.transpose(
                                tps[:], h_sb[:, c * 128 : (c + 1) * 128], ident[:]
                            )
                            nc.vector.tensor_copy(out_t[:, c, :], tps[:])

            h1t = pers.tile([128, 32, n_img], F16)
            fc_layer(w1s_d, 72,
                     lambda k: feat[:, k // 36, k % 36, :], fb1_d, h1t)
            h2t = pers.tile([128, 32, n_img], F16)
            fc_layer(w2s_d, 32, lambda k: h1t[:, k, :], fb2_d, h2t)

            # ============ gate + experts ============
            with tc.tile_pool(name="moe", bufs=1) as mp, \
                 tc.tile_pool(name="moeps", bufs=1, space="PSUM") as pp:
                ps_g = pp.tile([n_img, 4], F32)
                ps_e = pp.tile([n_img, 300], F32)
                for k in range(32):
                    nc.tensor.matmul(ps_g[:], h2t[:, k, :], gw[:, k, :],
                                     start=(k == 0), stop=(k == 31))
                for k in range(32):
                    nc.tensor.matmul(ps_e[:], h2t[:, k, :], ew[:, k, :],
                                     start=(k == 0), stop=(k == 31))

                sc = mp.tile([n_img, 4], F32)
                nc.vector.tensor_add(sc[:], ps_g[:], gb[:])
                mx = mp.tile([n_img, 1], F32)
                nc.vector.tensor_reduce(mx[:], sc[:], axis=mybir.AxisListType.X,
                                        op=mybir.AluOpType.max)
                m0 = mp.tile([n_img, 1], F32)
                m1 = mp.tile([n_img, 1], F32)
                m2 = mp.tile([n_img, 1], F32)
                t0 = mp.tile([n_img, 1], F32)
                nc.vector.tensor_tensor(m0[:], sc[:, 0:1], mx[:],
                                        mybir.AluOpType.is_ge)
                nc.vector.tensor_tensor(t0[:], sc[:, 1:2], mx[:],
                                        mybir.AluOpType.is_ge)
                # m0n = 1 - m0 ; m1 = t0 * m0n ; m2 = m0n - m1
                m0n = mp.tile([n_img, 1], F32)
                nc.vector.tensor_scalar(m0n[:], m0[:], -1.0, 1.0,
                                        mybir.AluOpType.mult, mybir.AluOpType.add)
                nc.vector.tensor_mul(m1[:], t0[:], m0n[:])
                nc.vector.tensor_sub(m2[:], m0n[:], m1[:])

                pe = mp.tile([n_img, 300], F32)
                nc.vector.tensor_add(pe[:], ps_e[:], eb[:])
                acc = mp.tile([n_img, 100], F32)
                nc.vector.tensor_scalar_mul(acc[:], pe[:, 0:100], m0[:])
                nc.vector.scalar_tensor_tensor(
                    acc[:], pe[:, 100:200], m1[:], acc[:],
                    op0=mybir.AluOpType.mult, op1=mybir.AluOpType.add)
                nc.vector.scalar_tensor_tensor(
                    acc[:], pe[:, 200:300], m2[:], acc[:],
                    op0=mybir.AluOpType.mult, op1=mybir.AluOpType.add)
                nc.sync.dma_start(out_d[:], acc[:])

    nc.finalize()
    return nc


_CACHE = {}


def kernel(**inputs):
    host = prep_host(inputs)
    n_img = BL
    if "nc" not in _CACHE:
        _CACHE["nc"] = build_nc(n_img)
    nc = _CACHE["nc"]

    shared = {k: v for k, v in host.items() if k != "xp"}
    in_maps = []
    for c in range(NCORE):
        m = dict(shared)
        m["xp"] = host["xp"][c * BL : (c + 1) * BL]
        in_maps.append(m)

    res = run_bass_kernel_spmd(
        nc, in_maps, core_ids=list(range(NCORE)),
        trace=bool(int(os.environ.get("KERNEL_TRACE", "0"))),
    )
    out = np.concatenate([res.results[c]["out"] for c in range(NCORE)], axis=0)
    if bool(int(os.environ.get("KERNEL_TRACE", "0"))):
        _CACHE["last_res"] = res
    return out


# revision 21
# speedup vs baseline: 1.1471x; 1.1471x over previous
"""AlexNet + top-1 MoE head, data-parallel over 8 TRN2 NeuronCores.

Self-contained: takes FULL inputs, shards batch 32/core, runs one SPMD
Bass kernel on cores 0-7, gathers [256, 100] output.
"""
import os
import ml_dtypes
import numpy as np

import concourse.bass as bass
import concourse.mybir as mybir
import concourse.tile as tile
from concourse import bacc
from concourse.bass_utils import run_bass_kernel_spmd
from concourse.masks import make_identity

F32 = mybir.dt.float32
F32R = mybir.dt.float32r
BF16 = mybir.dt.bfloat16
F16 = mybir.dt.float16

B = 256
NCORE = 8
BL = B // NCORE  # 32 images per core
BN_EPS = 1e-5

# ---------------------------------------------------------------------------
# host-side parameter/layout preparation (numpy only, no arithmetic on acts)
# ---------------------------------------------------------------------------


def _np(x):
    return np.asarray(x, dtype=np.float32)


def prep_host(inputs):
    conv_ws = [_np(w) for w in inputs["conv_ws"]]
    conv_bs = [_np(b) for b in inputs["conv_bs"]]
    bn_g = [_np(v) for v in inputs["bn_g"]]
    bn_b = [_np(v) for v in inputs["bn_b"]]
    bn_m = [_np(v) for v in inputs["bn_m"]]
    bn_v = [_np(v) for v in inputs["bn_v"]]

    w_eff, b_eff = [], []
    for i in range(5):
        sc = bn_g[i] / np.sqrt(bn_v[i] + BN_EPS)
        w_eff.append(conv_ws[i] * sc[:, None, None, None])
        b_eff.append(conv_bs[i] * sc + bn_b[i] - bn_m[i] * sc)

    out = {}

    # ---- conv1: phase decomposition of the stride-4 11x11 conv ----
    x = _np(inputs["x"])  # [B, 3, 227, 227]
    xp = np.zeros((B, 3, 228, 228), np.float32)
    xp[:, :, :227, :227] = x
    # phase planes: [B, ci, q, r, 57, 57]; partition index ciqr = ci*16+q*4+r
    ph = xp.reshape(B, 3, 57, 4, 57, 4).transpose(0, 1, 3, 5, 2, 4)
    php = np.zeros((B, 48, 3252), np.float32)
    php[:, :, :3249] = ph.reshape(B, 48, 3249)
    # replicated K-chunk layout: rows 0..127 = chunk0 (b0 all, b1 all, b2
    # ciqr 0..31); rows 128..175 = a-shifted b2 remainder (a*16 + (ciqr-32))
    xr = np.empty((B, 176, 3135), np.float32)
    xr[:, 0:48] = php[:, :, 0:3135]
    xr[:, 48:96] = php[:, :, 57:3192]
    xr[:, 96:128] = php[:, 0:32, 114:3249]
    for a in range(3):
        xr[:, 128 + 16 * a : 144 + 16 * a] = php[:, 32:48, 114 + a : 3249 + a]
    out["xp"] = xr.astype(np.float16)

    # w1 phase lhsT: [144 (b*48 + ciqr), 3 (a), 96 (co)]
    w1p = np.zeros((3, 48, 3, 96), np.float32)  # [b, ciqr, a, co]
    for b in range(3):
        for ci in range(3):
            for q in range(4):
                for r in range(4):
                    ky = 4 * b + q
                    if ky > 10:
                        continue
                    for a in range(3):
                        kx = 4 * a + r
                        if kx > 10:
                            continue
                        w1p[b, ci * 16 + q * 4 + r, a, :] = w_eff[0][:, ci, ky, kx]
    w1p = w1p.reshape(144, 3, 96)
    out["w1a"] = np.ascontiguousarray(w1p[0:128]).astype(np.float16)
    w1c = np.zeros((48, 96), np.float32)
    for a in range(3):
        w1c[a * 16 : (a + 1) * 16] = w1p[128:144, a, :]
    out["w1c"] = w1c.astype(np.float16)
    out["b1"] = b_eff[0].reshape(96, 1)

    # ---- conv2 lhsT: [480 (ky*96+ci), 5 (kx), 256 (co)] ----
    # w_eff[1]: [256, 96, 5, 5] -> [ky, ci, kx, co]
    out["w2p"] = np.ascontiguousarray(
        w_eff[1].transpose(2, 1, 3, 0).reshape(480, 5, 256)
    ).astype(np.float16)
    out["b2"] = b_eff[1].reshape(256, 1)

    # ---- conv3-5 lhsT: [ci, 9 (ky*3+kx), co] ----
    for i, nm in ((2, "w3p"), (3, "w4p"), (4, "w5p")):
        w = w_eff[i]  # [co, ci, 3, 3]
        out[nm] = np.ascontiguousarray(
            w.transpose(1, 2, 3, 0).reshape(w.shape[1], 9, w.shape[0])
        ).astype(np.float16)
    out["b3"] = b_eff[2].reshape(384, 1)
    out["b4"] = b_eff[3].reshape(384, 1)
    out["b5"] = b_eff[4].reshape(256, 1)

    # ---- fc1: K-order (cc, s, p): chunk k = cc*36+s, partition p = ch%128 ----
    fc1_w = _np(inputs["fc1_w"])  # [4096, 9216]
    w1s = fc1_w.T.reshape(2, 128, 36, 4096).transpose(0, 2, 1, 3)
    out["w1s"] = np.ascontiguousarray(w1s.reshape(72, 128, 4096)).astype(
        np.float16)
    out["fb1"] = np.broadcast_to(_np(inputs["fc1_b"]), (BL, 4096)).copy()

    fc2_w = _np(inputs["fc2_w"])  # [4096, 4096]
    out["w2s"] = np.ascontiguousarray(fc2_w.T.reshape(32, 128, 4096)).astype(
        np.float16)
    out["fb2"] = np.broadcast_to(_np(inputs["fc2_b"]), (BL, 4096)).copy()

    # ---- gate (padded to 4 experts) ----
    gate_w = _np(inputs["gate_w"])  # [3, 4096]
    gw4 = np.zeros((4, 4096), np.float32)
    gw4[:3] = gate_w
    out["gw"] = np.ascontiguousarray(gw4.T.reshape(32, 128, 4)).astype(np.float16)
    gb4 = np.full((4,), -1e30, np.float32)
    gb4[:3] = _np(inputs["gate_b"]) + _np(inputs["expert_bias"])
    out["gb"] = np.broadcast_to(gb4, (BL, 4)).copy()

    # ---- experts ----
    exp_w = _np(inputs["exp_w"])  # [3, 4096, 100]
    out["ew"] = np.ascontiguousarray(
        exp_w.transpose(1, 0, 2).reshape(32, 128, 300)
    ).astype(np.float16)
    out["eb"] = np.broadcast_to(
        _np(inputs["exp_b"]).reshape(300), (BL, 300)
    ).copy()
    return out


# ---------------------------------------------------------------------------
# device kernel
# ---------------------------------------------------------------------------


def memz(nc, ap):
    if mybir.dt.size(ap.dtype) == 2:
        nc.vector.memset(ap.bitcast(mybir.dt.uint16), 0)
    else:
        nc.vector.memset(ap.bitcast(mybir.dt.uint32), 0)


def rview(t, off, dims):
    """Strided free-dim view of an SBUF tile AP (keeps its partition dim)."""
    base = t if isinstance(t, bass.AP) else t[:]
    part = list(base.ap[0])
    return bass.AP(base.tensor, base.offset + off, [part] + [list(d) for d in dims])


def build_nc(n_img=BL):
    nc = bacc.Bacc("TRN2", target_bir_lowering=False, debug=False)
    AF = mybir.ActivationFunctionType

    xp_d = nc.dram_tensor("xp", [n_img, 176, 3135], F16, kind="ExternalInput")
    w1a_d = nc.dram_tensor("w1a", [128, 3, 96], F16, kind="ExternalInput")
    w1c_d = nc.dram_tensor("w1c", [48, 96], F16, kind="ExternalInput")
    b1_d = nc.dram_tensor("b1", [96, 1], F32, kind="ExternalInput")
    w2p_d = nc.dram_tensor("w2p", [480, 5, 256], F16, kind="ExternalInput")
    b2_d = nc.dram_tensor("b2", [256, 1], F32, kind="ExternalInput")
    w3p_d = nc.dram_tensor("w3p", [256, 9, 384], F16, kind="ExternalInput")
    b3_d = nc.dram_tensor("b3", [384, 1], F32, kind="ExternalInput")
    w4p_d = nc.dram_tensor("w4p", [384, 9, 384], F16, kind="ExternalInput")
    b4_d = nc.dram_tensor("b4", [384, 1], F32, kind="ExternalInput")
    w5p_d = nc.dram_tensor("w5p", [384, 9, 256], F16, kind="ExternalInput")
    b5_d = nc.dram_tensor("b5", [256, 1], F32, kind="ExternalInput")
    w1s_d = nc.dram_tensor("w1s", [72, 128, 4096], F16, kind="ExternalInput")
    fb1_d = nc.dram_tensor("fb1", [n_img, 4096], F32, kind="ExternalInput")
    w2s_d = nc.dram_tensor("w2s", [32, 128, 4096], F16, kind="ExternalInput")
    fb2_d = nc.dram_tensor("fb2", [n_img, 4096], F32, kind="ExternalInput")
    gw_d = nc.dram_tensor("gw", [32, 128, 4], F16, kind="ExternalInput")
    gb_d = nc.dram_tensor("gb", [n_img, 4], F32, kind="ExternalInput")
    ew_d = nc.dram_tensor("ew", [32, 128, 300], F16, kind="ExternalInput")
    eb_d = nc.dram_tensor("eb", [n_img, 300], F32, kind="ExternalInput")
    out_d = nc.dram_tensor("out", [n_img, 100], F32, kind="ExternalOutput")

    act1_d = nc.dram_tensor("act1", [n_img, 96, 961], F16)
    act2_d = nc.dram_tensor("act2", [n_img, 256, 225], F16)
    act3_d = nc.dram_tensor("act3", [n_img, 384, 225], F16)
    act4_d = nc.dram_tensor("act4", [n_img, 384, 225], F16)

    assert n_img % 2 == 0
    npair = n_img // 2

    with tile.TileContext(nc) as tc:
        with tc.tile_pool(name="persist", bufs=1) as pers:
            # features [128, cc, s, img] gathered across the conv phase
            feat = pers.tile([128, 2, 36, n_img], F16)

            # all conv weights+biases preloaded on the gpsimd DMA queue so
            # they never stall the sync queue at phase boundaries
            cw = pers
            w1a = cw.tile([128, 3, 96], F16)
            nc.gpsimd.dma_start(w1a[:], w1a_d[:])
            w1c = cw.tile([48, 96], F16)
            nc.gpsimd.dma_start(w1c[:], w1c_d[:])
            b1 = cw.tile([96, 1], F32)
            nc.gpsimd.dma_start(b1[:], b1_d[:])
            w2c = []
            for t in range(4):
                kk = 128 if t < 3 else 96
                w2t = cw.tile([kk, 5, 256], F16, tag=f"w2_{t}", name=f"w2_{t}")
                nc.gpsimd.dma_start(w2t[:], w2p_d[t * 128 : t * 128 + kk])
                w2c.append(w2t)
            b2a = cw.tile([128, 1], F32)
            nc.gpsimd.dma_start(b2a[:], b2_d[0:128])
            b2b = cw.tile([128, 1], F32)
            nc.gpsimd.dma_start(b2b[:], b2_d[128:256])

            def load_w33(w_d, b_d, nci, nco, pfx):
                wts, bs = [], []
                for t in range(nci // 128):
                    wt = cw.tile([128, 9, nco], F16, tag=f"{pfx}w_{t}",
                                 name=f"{pfx}w_{t}")
                    nc.gpsimd.dma_start(wt[:], w_d[t * 128 : (t + 1) * 128])
                    wts.append(wt)
                for c in range(nco // 128):
                    bt = cw.tile([128, 1], F32, tag=f"{pfx}b_{c}", name=f"{pfx}b_{c}")
                    nc.gpsimd.dma_start(bt[:], b_d[c * 128 : (c + 1) * 128])
                    bs.append(bt)
                return wts, bs

            w3t, b3t = load_w33(w3p_d, b3_d, 256, 384, "c3")
            w4t, b4t = load_w33(w4p_d, b4_d, 384, 384, "c4")
            w5t, b5t = load_w33(w5p_d, b5_d, 384, 256, "c5")

            gw = cw.tile([128, 32, 4], F16)
            nc.gpsimd.dma_start(gw[:], gw_d[:].rearrange("k p e -> p k e"))
            ew = cw.tile([128, 32, 300], F16)
            nc.gpsimd.dma_start(ew[:], ew_d[:].rearrange("k p e -> p k e"))
            gb = cw.tile([n_img, 4], F32)
            nc.gpsimd.dma_start(gb[:], gb_d[:])
            eb = cw.tile([n_img, 300], F32)
            nc.gpsimd.dma_start(eb[:], eb_d[:])

            # ============ conv1 ============
            with tc.tile_pool(name="c1x", bufs=2) as xpool, \
                 tc.tile_pool(name="c1o", bufs=2) as opool, \
                 tc.tile_pool(name="c1ps", bufs=4, space="PSUM") as pp:
                for im in range(n_img):
                    x1a = xpool.tile([128, 3152], F16, tag="x1a")
                    x1c = xpool.tile([48, 3152], F16, tag="x1c")
                    nc.sync.dma_start(x1a[:, 0:3135], xp_d[im, 0:128])
                    nc.sync.dma_start(x1c[:, 0:3135], xp_d[im, 128:176])

                    c1o = opool.tile([96, 55, 56], F16, tag="c1o")
                    y0 = 0
                    while y0 < 55:
                        ny = min(9, 55 - y0)
                        ps = pp.tile([96, 512], F32, tag="ps", name="ps")[:, : ny * 56]
                        for a in range(3):
                            nc.tensor.matmul(
                                ps,
                                w1a[:, a, :],
                                rview(x1a, y0 * 57 + a, [[57, ny], [1, 56]]),
                                start=(a == 0), stop=False,
                            )
                        nc.tensor.matmul(
                            ps,
                            w1c[:],
                            rview(x1c, y0 * 57, [[57, ny], [1, 56]]),
                            start=False, stop=True,
                        )
                        nc.scalar.activation(
                            c1o[:, y0 : y0 + ny, :],
                            ps.rearrange("p (y x) -> p y x", y=ny),
                            AF.Relu, bias=b1[:], scale=1.0,
                        )
                        y0 += ny

                    # maxpool 3x3 s2: 55x55 -> 27x27 (col 55 of c1o is garbage)
                    t1 = opool.tile([96, 55, 27], F16, tag="t1")
                    v = c1o[:]
                    nc.vector.tensor_max(t1[:], v[:, :, 0:54:2], v[:, :, 1:55:2])
                    nc.vector.tensor_max(t1[:], t1[:], v[:, :, 2:56:2])
                    tmp2 = opool.tile([96, 27, 27], F16, tag="tmp2")
                    nc.vector.tensor_max(tmp2[:], t1[:, 0:54:2, :], t1[:, 1:55:2, :])
                    pad1 = opool.tile([96, 961], F16, tag="pad1")
                    memz(nc, pad1[:])
                    p1v = pad1[:].rearrange("p (y x) -> p y x", y=31)
                    nc.vector.tensor_max(p1v[:, 2:29, 2:29], tmp2[:], t1[:, 2:55:2, :])
                    nc.sync.dma_start(act1_d[im], pad1[:])

            # ============ conv2 ============
            # K enumeration p = ky*96 + ci over [0,480); chunks of 128.
            # sections: (chunk, dst_lo, dst_hi, ky, ci_lo, ci_hi)
            sects = [
                (0, 0, 96, 0, 0, 96), (0, 96, 128, 1, 0, 32),
                (1, 0, 64, 1, 32, 96), (1, 64, 128, 2, 0, 64),
                (2, 0, 32, 2, 64, 96), (2, 32, 128, 3, 0, 96),
                (3, 0, 96, 4, 0, 96),
            ]
            with tc.tile_pool(name="c2x", bufs=2) as xpool, \
                 tc.tile_pool(name="c2o", bufs=2) as opool, \
                 tc.tile_pool(name="c2ps", bufs=4, space="PSUM") as pp:
                for pr in range(npair):
                    im = 2 * pr
                    k2 = []
                    for t in range(4):
                        kk = 128 if t < 3 else 96
                        k2t = xpool.tile([kk, 1696], F16, tag=f"k2_{t}", name=f"k2_{t}")
                        k2.append(k2t)
                    for (t, dlo, dhi, ky, clo, chi) in sects:
                        nc.sync.dma_start(
                            rview(k2[t][dlo:dhi], 0, [[844, 2], [1, 837]]),
                            act1_d[im : im + 2, clo:chi, 31 * ky : 31 * ky + 837]
                            .rearrange("n k f -> k n f"),
                        )
                    for cc in range(2):
                        bia = b2a[:] if cc == 0 else b2b[:]
                        c2o = opool.tile([128, 2, 27, 27], F16, tag=f"c2o_{cc}",
                                         name=f"c2o_{cc}")
                        for y0 in (0, 9, 18):
                            ps = pp.tile([128, 486], F32, tag="ps", name="ps")
                            first = True
                            for t in range(4):
                                for kx in range(5):
                                    nc.tensor.matmul(
                                        ps[:],
                                        w2c[t][:, kx, cc * 128 : cc * 128 + 128],
                                        rview(k2[t], y0 * 31 + kx,
                                              [[844, 2], [31, 9], [1, 27]]),
                                        start=first,
                                        stop=(t == 3 and kx == 4),
                                    )
                                    first = False
                            nc.scalar.activation(
                                c2o[:, :, y0 : y0 + 9, :],
                                ps[:].rearrange("p (n y x) -> p n y x", n=2, y=9),
                                AF.Relu, bias=bia, scale=1.0,
                            )
                        # pool 27x27 -> 13x13 (col 27 garbage)
                        t1 = opool.tile([128, 2, 27, 13], F16, tag="p2t1")
                        v = c2o[:]
                        nc.vector.tensor_max(t1[:], v[:, :, :, 0:25:2], v[:, :, :, 1:26:2])
                        nc.vector.tensor_max(t1[:], t1[:], v[:, :, :, 2:27:2])
                        tmp2 = opool.tile([128, 2, 13, 13], F16, tag="p2tmp2")
                        nc.vector.tensor_max(tmp2[:], t1[:, :, 0:25:2, :],
                                             t1[:, :, 1:26:2, :])
                        pad2 = opool.tile([128, 2, 225], F16, tag="pad2")
                        memz(nc, pad2[:])
                        p2v = pad2[:].rearrange("p n (y x) -> p n y x", y=15)
                        nc.vector.tensor_max(p2v[:, :, 1:14, 1:14], tmp2[:],
                                             t1[:, :, 2:27:2, :])
                        nc.sync.dma_start(
                            act2_d[im : im + 2, cc * 128 : cc * 128 + 128, :]
                            .rearrange("n k f -> k n f"),
                            pad2[:],
                        )

            # ============ conv3 / conv4 / conv5 (pairs of images) ============
            def conv33(src_d, dst_d, wts, bs, nci, nco, pool5=False):
                """3x3 pad-1 conv on 15x15-padded 13x13 acts, 2 imgs/pass."""
                ncic = nci // 128
                ncoc = nco // 128
                with tc.tile_pool(name="c3x", bufs=2) as xpool, \
                     tc.tile_pool(name="c3o", bufs=2) as opool, \
                     tc.tile_pool(name="c3ps", bufs=4, space="PSUM") as pp:
                    for pr in range(npair):
                        im = 2 * pr
                        xt = []
                        for t in range(ncic):
                            x3 = xpool.tile([128, 456], F16, tag=f"x_{t}",
                                            name=f"x_{t}")
                            nc.sync.dma_start(
                                rview(x3, 0, [[226, 2], [1, 225]]),
                                src_d[im : im + 2, t * 128 : (t + 1) * 128, :]
                                .rearrange("n k f -> k n f"),
                            )
                            xt.append(x3)
                        for c in range(ncoc):
                            ps = pp.tile([128, 338], F32, tag="ps")
                            first = True
                            for t in range(ncic):
                                for ky in range(3):
                                    for kx in range(3):
                                        nc.tensor.matmul(
                                            ps[:],
                                            wts[t][:, 3 * ky + kx,
                                                   c * 128 : (c + 1) * 128],
                                            rview(xt[t], ky * 15 + kx,
                                                  [[226, 2], [15, 13], [1, 13]]),
                                            start=first,
                                            stop=(t == ncic - 1 and ky == 2
                                                  and kx == 2),
                                        )
                                        first = False
                            if not pool5:
                                pad3 = opool.tile([128, 2, 225], F16,
                                                  tag=f"pad_{c}", name=f"pad_{c}")
                                memz(nc, pad3[:])
                                p3v = pad3[:].rearrange("p n (y x) -> p n y x", y=15)
                                nc.scalar.activation(
                                    p3v[:, :, 1:14, 1:14],
                                    ps[:].rearrange("p (n y x) -> p n y x",
                                                    n=2, y=13),
                                    AF.Relu, bias=bs[c], scale=1.0,
                                )
                                nc.sync.dma_start(
                                    dst_d[im : im + 2, c * 128 : (c + 1) * 128, :]
                                    .rearrange("n k f -> k n f"),
                                    pad3[:],
                                )
                            else:
                                c5o = opool.tile([128, 2, 13, 13], F16,
                                                 tag=f"c5o_{c}", name=f"c5o_{c}")
                                nc.scalar.activation(
                                    c5o[:],
                                    ps[:].rearrange("p (n y x) -> p n y x",
                                                    n=2, y=13),
                                    AF.Relu, bias=bs[c], scale=1.0,
                                )
                                # pool 13x13 -> 6x6 into feat[:, c, :, im:im+2]
                                t1 = opool.tile([128, 2, 13, 6], F16, tag="p5t1")
                                v = c5o[:]
                                nc.vector.tensor_max(t1[:], v[:, :, :, 0:11:2],
                                                     v[:, :, :, 1:12:2])
                                nc.vector.tensor_max(t1[:], t1[:], v[:, :, :, 2:13:2])
                                tmp2 = opool.tile([128, 2, 6, 6], F16, tag="p5tmp2")
                                nc.vector.tensor_max(tmp2[:], t1[:, :, 0:11:2, :],
                                                     t1[:, :, 1:12:2, :])
                                fv = feat[:].rearrange(
                                    "p c (y x) n -> p c y x n", y=6)
                                for j in range(2):
                                    nc.vector.tensor_max(
                                        fv[:, c, :, :, im + j],
                                        tmp2[:, j], t1[:, j, 2:13:2, :]
                                    )

            conv33(act2_d, act3_d, w3t, b3t, 256, 384)
            conv33(act3_d, act4_d, w4t, b4t, 384, 384)
            conv33(act4_d, None, w5t, b5t, 384, 256, pool5=True)

            # ============ FC trunk ============
            ident = pers.tile([n_img, n_img], F32)
            make_identity(nc, ident[:])

            def fc_layer(w_stream_d, nk, lhs_get, bias_d, out_t):
                """out = relu(lhs.T @ W + b); lhs chunks [128, n_img] stationary.
                Writes transposed f32r result into out_t [128, nk_out, n_img]."""
                with tc.tile_pool(name="fcw", bufs=8) as wpool, \
                     tc.tile_pool(name="fco", bufs=1) as opool:
                    bias = opool.tile([n_img, 4096], F32)
                    nc.sync.dma_start(bias[:], bias_d[:])
                    h_sb = opool.tile([n_img, 4096], F32)
                    with tc.tile_pool(name="fcps", bufs=1, space="PSUM") as pp:
                        pss = [pp.tile([n_img, 512], F32, name=f"fps_{n}")
                               for n in range(8)]
                        for k in range(nk):
                            wt = wpool.tile([128, 4096], F16, tag="wt")
                            nc.sync.dma_start(wt[:], w_stream_d[k])
                            lhs = lhs_get(k)
                            for n in range(8):
                                nc.tensor.matmul(
                                    pss[n][:], lhs, wt[:, n * 512 : (n + 1) * 512],
                                    start=(k == 0), stop=(k == nk - 1),
                                )
                        for n in range(8):
                            sl = slice(n * 512, (n + 1) * 512)
                            nc.vector.tensor_add(h_sb[:, sl], pss[n][:], bias[:, sl])
                            nc.vector.tensor_scalar_max(h_sb[:, sl], h_sb[:, sl], 0.0)
                    with tc.tile_pool(name="fct", bufs=2, space="PSUM") as tp:
                        for c in range(32):
                            tps = tp.tile([128, n_img], F32, tag="tps")
                            nc.tensor.transpose(
                                tps[:], h_sb[:, c * 128 : (c + 1) * 128], ident[:]
                            )
                            nc.vector.tensor_copy(out_t[:, c, :], tps[:])

            h1t = pers.tile([128, 32, n_img], F16)
            fc_layer(w1s_d, 72,
                     lambda k: feat[:, k // 36, k % 36, :], fb1_d, h1t)
            h2t = pers.tile([128, 32, n_img], F16)
            fc_layer(w2s_d, 32, lambda k: h1t[:, k, :], fb2_d, h2t)

            # ============ gate + experts ============
            with tc.tile_pool(name="moe", bufs=1) as mp, \
                 tc.tile_pool(name="moeps", bufs=1, space="PSUM") as pp:
                ps_g = pp.tile([n_img, 4], F32)
                ps_e = pp.tile([n_img, 300], F32)
                for k in range(32):
                    nc.tensor.matmul(ps_g[:], h2t[:, k, :], gw[:, k, :],
                                     start=(k == 0), stop=(k == 31))
                for k in range(32):
                    nc.tensor.matmul(ps_e[:], h2t[:, k, :], ew[:, k, :],
                                     start=(k == 0), stop=(k == 31))

                sc = mp.tile([n_img, 4], F32)
                nc.vector.tensor_add(sc[:], ps_g[:], gb[:])
                mx = mp.tile([n_img, 1], F32)
                nc.vector.tensor_reduce(mx[:], sc[:], axis=mybir.AxisListType.X,
                                        op=mybir.AluOpType.max)
                m0 = mp.tile([n_img, 1], F32)
                m1 = mp.tile([n_img, 1], F32)
                m2 = mp.tile([n_img, 1], F32)
                t0 = mp.tile([n_img, 1], F32)
                nc.vector.tensor_tensor(m0[:], sc[:, 0:1], mx[:],
                                        mybir.AluOpType.is_ge)
                nc.vector.tensor_tensor(t0[:], sc[:, 1:2], mx[:],
                                        mybir.AluOpType.is_ge)
                # m0n = 1 - m0 ; m1 = t0 * m0n ; m2 = m0n - m1
                m0n = mp.tile([n_img, 1], F32)
                nc.vector.tensor_scalar(m0n[:], m0[:], -1.0, 1.0,
                                        mybir.AluOpType.mult, mybir.AluOpType.add)
                nc.vector.tensor_mul(m1[:], t0[:], m0n[:])
                nc.vector.tensor_sub(m2[:], m0n[:], m1[:])

                pe = mp.tile([n_img, 300], F32)
                nc.vector.tensor_add(pe[:], ps_e[:], eb[:])
                acc = mp.tile([n_img, 100], F32)
                nc.vector.tensor_scalar_mul(acc[:], pe[:, 0:100], m0[:])
                nc.vector.scalar_tensor_tensor(
                    acc[:], pe[:, 100:200], m1[:], acc[:],
                    op0=mybir.AluOpType.mult, op1=mybir.AluOpType.add)
                nc.vector.scalar_tensor_tensor(
                    acc[:], pe[:, 200:300], m2[:], acc[:],
                    op0=mybir.AluOpType.mult, op1=mybir.AluOpType.add)
                nc.sync.dma_start(out_d[:], acc[:])

    nc.finalize()
    return nc


_CACHE = {}


def kernel(**inputs):
    host = prep_host(inputs)
    n_img = BL
    if "nc" not in _CACHE:
        _CACHE["nc"] = build_nc(n_img)
    nc = _CACHE["nc"]

    shared = {k: v for k, v in host.items() if k != "xp"}
    in_maps = []
    for c in range(NCORE):
        m = dict(shared)
        m["xp"] = host["xp"][c * BL : (c + 1) * BL]
        in_maps.append(m)

    res = run_bass_kernel_spmd(
        nc, in_maps, core_ids=list(range(NCORE)),
        trace=bool(int(os.environ.get("KERNEL_TRACE", "0"))),
    )
    out = np.concatenate([res.results[c]["out"] for c in range(NCORE)], axis=0)
    if bool(int(os.environ.get("KERNEL_TRACE", "0"))):
        _CACHE["last_res"] = res
    return out
